# revision 5
# baseline (speedup 1.0000x reference)
"""MegrezMoE MoE layer on 8 Trainium2 NeuronCores.

Strategy (expert-parallel, host-routed dispatch):
  - Host computes the (tiny) router: logits -> grouped top-k ids/weights,
    exactly mirroring the reference's noaux_tc selection.
  - 32 experts are assigned 4-per-core, balanced by routed-token count.
    Tokens are gathered per expert on the host (transposed: [H, rows],
    rows padded to a per-slot static capacity) so the device kernel is a
    fully static SPMD program: per expert slot, gate_up matmul ->
    silu*mul -> down matmul, streaming the expert weight bank from HBM
    exactly once per core.
  - The shared-expert MLP is tensor-parallel across the 8 cores (each
    core owns a 128-wide slice of the shared intermediate dim) and its
    partial outputs are summed on the host.
  - Matmuls run in bf16 with fp32 PSUM accumulation (KERNEL_DTYPE=f32r
    selects a float32r variant that keeps fp32 data in HBM).
  - Host combines: out[t] = sum_k w[t,k]*SCALE * y_col(t,k) + shared[t].

kernel() takes the full unsharded inputs, returns the full [1024, 2048]
fp32 output.
"""

import os

import ml_dtypes
import numpy as np

import concourse.bass as bass
import concourse.tile as tile
from concourse import bacc, mybir
from concourse.bass_utils import run_bass_kernel_spmd

# Model dims (hardcoded per problem spec)
H = 2048
E = 32
I = 1024
TOPK = 6
NGROUP = 8
TOPKG = 4
SCALE = 2.5
T = 1024

N_CORES = 8
EPC = 4          # experts per core
KT_H = H // 128  # 16 k-tiles over hidden dim
KT_I = I // 128  # 8 k-tiles over intermediate dim
WLOAD_K = 4      # k-tiles per weight DMA

F32 = mybir.dt.float32
F32R = mybir.dt.float32r
BF16 = mybir.dt.bfloat16

_PROGRAM_CACHE = {}
LAST_RESULTS = None  # BassKernelResults from the most recent run (for harness)


def _mode():
    return os.environ.get("KERNEL_DTYPE", "bf16")


# ---------------------------------------------------------------------------
# Host-side routing (mirrors reference._grouped_topk in fp32 numpy)
# ---------------------------------------------------------------------------

def _host_routing(x, gate_w, e_bias):
    logits = x @ gate_w                                   # [T, E] fp32
    scores = 1.0 / (1.0 + np.exp(-logits, dtype=np.float32))
    scores_choice = scores + e_bias[None, :]
    gsize = E // NGROUP
    grp = scores_choice.reshape(T, NGROUP, gsize)
    top2 = np.sort(grp, axis=-1)[:, :, -2:]
    group_scores = top2.sum(-1)                           # [T, G]
    gidx = np.argsort(-group_scores, axis=-1, kind="stable")[:, :TOPKG]
    gmask = np.zeros((T, NGROUP), bool)
    np.put_along_axis(gmask, gidx, True, axis=1)
    emask = np.repeat(gmask, gsize, axis=1)
    masked = np.where(emask, scores_choice, -np.inf)
    topk_ids = np.argsort(-masked, axis=-1, kind="stable")[:, :TOPK]
    topk_w = np.take_along_axis(scores, topk_ids, axis=1)
    topk_w = topk_w / topk_w.sum(-1, keepdims=True)
    return topk_w.astype(np.float32), topk_ids.astype(np.int64)


# ---------------------------------------------------------------------------
# Dispatch plan: expert -> (core, slot), per-slot static row capacities
# ---------------------------------------------------------------------------

def _make_plan(topk_ids):
    counts = np.bincount(topk_ids.ravel(), minlength=E)
    # blocks of 128 rows per expert; min 2 blocks keeps matmul N>=256
    blocks = np.maximum(2, np.ceil(counts / 128).astype(int))
    order = np.argsort(-blocks, kind="stable")            # experts, big first
    expert_of = np.zeros((N_CORES, EPC), dtype=int)
    slot_rows = []
    for s in range(EPC):
        chunk = order[s * N_CORES:(s + 1) * N_CORES]
        for c in range(N_CORES):
            expert_of[c, s] = chunk[c]
        slot_rows.append(int(blocks[chunk].max()) * 128)
    offs = np.concatenate([[0], np.cumsum(slot_rows)])
    return {
        "expert_of": expert_of,
        "slot_rows": tuple(slot_rows),
        "slot_offs": offs[:-1],
        "rtotal": int(offs[-1]),
        "counts": counts,
    }


# ---------------------------------------------------------------------------
# Bass program (SPMD; one program, per-core data)
# ---------------------------------------------------------------------------

def _build_program(slot_rows, mode):
    rtotal = sum(slot_rows)
    f32r = mode == "f32r"
    DTD = F32 if f32r else BF16      # dram dtype of matmul operands
    DTS = F32R if f32r else BF16     # sbuf dtype of matmul operands

    nc = bacc.Bacc("TRN2", target_bir_lowering=False, debug=False,
                   num_devices=N_CORES)

    # DRAM I/O (per core). Weight banks arrive host-reordered into k-slab
    # layout so every DMA is a large contiguous read:
    #   wgu: [EPC, 2(gate/up), KT_H, 128, I]
    #   wd:  [EPC, 2(out half), KT_I, 128, I]
    xg = nc.dram_tensor("xg", [H, rtotal], DTD, kind="ExternalInput")
    xf = nc.dram_tensor("xf", [H, T], DTD, kind="ExternalInput")
    wgu = nc.dram_tensor("wgu", [EPC, 2, KT_H, 128, I], DTD, kind="ExternalInput")
    wd = nc.dram_tensor("wd", [EPC, 2, KT_I, 128, I], DTD, kind="ExternalInput")
    wsg = nc.dram_tensor("wsg", [KT_H, 128, 256], DTD, kind="ExternalInput")
    wsd = nc.dram_tensor("wsd", [128, H], DTD, kind="ExternalInput")
    yr = nc.dram_tensor("yr", [H, rtotal], F32, kind="ExternalOutput")
    ys = nc.dram_tensor("ys", [H, T], F32, kind="ExternalOutput")

    # casting DMA (fp32 dram -> f32r sbuf) must go via SWDGE
    ldma = nc.gpsimd.dma_start if f32r else nc.sync.dma_start

    def src3(ap2d, a):
        """DRAM [(a p), m] view as [p, a, m] for tiled SBUF loads."""
        return ap2d.rearrange("(a p) m -> a p m", p=128).transpose([1, 0, 2])

    def sb3(tile_ap, a):
        """SBUF [p, (a m)] view as [p, a, m]."""
        return tile_ap.rearrange("p (a m) -> p a m", a=a)

    with tile.TileContext(nc) as tc:
        with tc.tile_pool(name="psum", bufs=8, space="PSUM") as psum_pool:
            # ---------------- shared expert (TP slice) ----------------
            with tc.tile_pool(name="swg", bufs=1) as swg_pool, \
                 tc.tile_pool(name="swd", bufs=1) as swd_pool, \
                 tc.tile_pool(name="sxf", bufs=2) as sxf_pool, \
                 tc.tile_pool(name="sact", bufs=2) as sact_pool, \
                 tc.tile_pool(name="sout", bufs=2) as sout_pool:
                wsg_sb = swg_pool.tile([128, KT_H * 256], DTS)
                ldma(sb3(wsg_sb[:], KT_H), wsg.ap().transpose([1, 0, 2]))
                wsd_sb = swd_pool.tile([128, H], DTS)
                ldma(wsd_sb[:], wsd.ap())

                CH = 256
                for ch in range(T // CH):
                    xf_sb = sxf_pool.tile([128, KT_H * CH], DTS)
                    ldma(sb3(xf_sb[:], KT_H),
                         src3(xf.ap()[:, ch * CH:(ch + 1) * CH], KT_H))
                    ps_g = psum_pool.tile([128, CH], F32, tag="ps")
                    ps_u = psum_pool.tile([128, CH], F32, tag="ps")
                    for k in range(KT_H):
                        lg = wsg_sb[:, k * 256:k * 256 + 128]
                        lu = wsg_sb[:, k * 256 + 128:k * 256 + 256]
                        rx = xf_sb[:, k * CH:(k + 1) * CH]
                        nc.tensor.matmul(ps_g[:], lg, rx,
                                         start=(k == 0), stop=(k == KT_H - 1))
                        nc.tensor.matmul(ps_u[:], lu, rx,
                                         start=(k == 0), stop=(k == KT_H - 1))
                    gs = sact_pool.tile([128, CH], F32, tag="sgs")
                    nc.scalar.activation(gs[:], ps_g[:],
                                         mybir.ActivationFunctionType.Sigmoid)
                    nc.vector.tensor_mul(gs[:], gs[:], ps_g[:])
                    a_s = sact_pool.tile([128, CH], DTS, tag="sas")
                    nc.vector.tensor_mul(a_s[:], gs[:], ps_u[:])
                    # down: 16 output m-tiles, single k (the 128-slice of I)
                    for half in range(2):
                        stg = sout_pool.tile([128, 8 * CH], F32, tag="sstg")
                        for m in range(8):
                            pd = psum_pool.tile([128, CH], F32, tag="ps",
                                                name="pd")
                            lw = wsd_sb[:, (half * 8 + m) * 128:
                                        (half * 8 + m + 1) * 128]
                            nc.tensor.matmul(pd[:], lw, a_s[:],
                                             start=True, stop=True)
                            nc.scalar.copy(stg[:, m * CH:(m + 1) * CH], pd[:])
                        nc.sync.dma_start(
                            src3(ys.ap()[half * 1024:(half + 1) * 1024,
                                         ch * CH:(ch + 1) * CH], 8),
                            sb3(stg[:], 8))

            # ---------------- routed experts ----------------
            with tc.tile_pool(name="wsl", bufs=3) as w_pool, \
                 tc.tile_pool(name="xs", bufs=2) as x_pool, \
                 tc.tile_pool(name="gs", bufs=2) as g_pool, \
                 tc.tile_pool(name="at", bufs=2) as a_pool, \
                 tc.tile_pool(name="ost", bufs=2) as o_pool:
                off = 0
                for s in range(EPC):
                    R = slot_rows[s]
                    xs = x_pool.tile([128, KT_H * R], DTS, tag="xs")
                    ldma(sb3(xs[:], KT_H), src3(xg.ap()[:, off:off + R], KT_H))

                    gs = g_pool.tile([128, KT_I * R], F32, tag="gs")
                    at = a_pool.tile([128, KT_I * R], DTS, tag="at")

                    for phase in range(2):  # 0 = gate, 1 = up
                        ps = [psum_pool.tile([128, R], F32, tag="ps", name="ps")
                              for _ in range(8)]
                        for kb in range(KT_H // WLOAD_K):
                            wt = w_pool.tile([128, WLOAD_K * I], DTS, tag="wsl")
                            ldma(sb3(wt[:], WLOAD_K),
                                 wgu.ap()[s, phase,
                                          kb * WLOAD_K:(kb + 1) * WLOAD_K]
                                 .transpose([1, 0, 2]))
                            for kk in range(WLOAD_K):
                                k = kb * WLOAD_K + kk
                                rx = xs[:, k * R:(k + 1) * R]
                                for m in range(8):
                                    lw = wt[:, kk * I + m * 128:
                                            kk * I + (m + 1) * 128]
                                    nc.tensor.matmul(
                                        ps[m][:], lw, rx,
                                        start=(k == 0), stop=(k == KT_H - 1))
                        for m in range(8):
                            if phase == 0:
                                nc.scalar.activation(
                                    gs[:, m * R:(m + 1) * R], ps[m][:],
                                    mybir.ActivationFunctionType.Sigmoid)
                                nc.vector.tensor_mul(
                                    gs[:, m * R:(m + 1) * R],
                                    gs[:, m * R:(m + 1) * R], ps[m][:])
                            else:
                                nc.vector.tensor_mul(
                                    at[:, m * R:(m + 1) * R],
                                    gs[:, m * R:(m + 1) * R], ps[m][:])

                    for half in range(2):
                        ps = [psum_pool.tile([128, R], F32, tag="ps", name="ps")
                              for _ in range(8)]
                        for kb in range(KT_I // WLOAD_K):
                            wt = w_pool.tile([128, WLOAD_K * I], DTS, tag="wsl")
                            ldma(sb3(wt[:], WLOAD_K),
                                 wd.ap()[s, half,
                                         kb * WLOAD_K:(kb + 1) * WLOAD_K]
                                 .transpose([1, 0, 2]))
                            for kk in range(WLOAD_K):
                                k = kb * WLOAD_K + kk
                                ra = at[:, k * R:(k + 1) * R]
                                for m in range(8):
                                    lw = wt[:, kk * I + m * 128:
                                            kk * I + (m + 1) * 128]
                                    nc.tensor.matmul(
                                        ps[m][:], lw, ra,
                                        start=(k == 0), stop=(k == KT_I - 1))
                        stg = o_pool.tile([128, 8 * R], F32, tag="ost")
                        for m in range(8):
                            nc.scalar.copy(stg[:, m * R:(m + 1) * R], ps[m][:])
                        nc.sync.dma_start(
                            src3(yr.ap()[half * 1024:(half + 1) * 1024,
                                         off:off + R], 8),
                            sb3(stg[:], 8))
                    off += R

    nc.compile()
    return nc


def _get_program(slot_rows, mode):
    key = (tuple(slot_rows), mode)
    if key not in _PROGRAM_CACHE:
        _PROGRAM_CACHE[key] = _build_program(slot_rows, mode)
    return _PROGRAM_CACHE[key]


# ---------------------------------------------------------------------------
# Per-core input construction (host shard + reorder + cast)
# ---------------------------------------------------------------------------

def _make_in_maps(x, w_gate_up, w_down, shared_gate_up, shared_down,
                  topk_ids, plan, mode):
    rtotal = plan["rtotal"]
    offs = plan["slot_offs"]
    expert_of = plan["expert_of"]
    np_dt = np.float32 if mode == "f32r" else ml_dtypes.bfloat16

    tok_of = [np.where((topk_ids == e).any(axis=1))[0] for e in range(E)]
    flat_col = np.zeros((T, TOPK), dtype=np.int64)

    xT = np.ascontiguousarray(x.T).astype(np_dt)          # [H, T]
    wgu_r = w_gate_up.reshape(E, KT_H, 128, 2, I).transpose(0, 3, 1, 2, 4)
    wd_r = w_down.reshape(E, KT_I, 128, 2, I).transpose(0, 3, 1, 2, 4)

    in_maps = []
    for c in range(N_CORES):
        xg_c = np.zeros((H, rtotal), dtype=np_dt)
        for s in range(EPC):
            e = expert_of[c, s]
            toks = tok_of[e]
            xg_c[:, offs[s]:offs[s] + len(toks)] = xT[:, toks]
            col_base = c * rtotal + offs[s]
            for pos, t in enumerate(toks):
                for k in np.nonzero(topk_ids[t] == e)[0]:
                    flat_col[t, k] = col_base + pos
        sl = slice(c * 128, (c + 1) * 128)
        in_maps.append({
            "xg": xg_c,
            "xf": xT,
            "wgu": np.ascontiguousarray(wgu_r[expert_of[c]]).astype(np_dt),
            "wd": np.ascontiguousarray(wd_r[expert_of[c]]).astype(np_dt),
            "wsg": np.ascontiguousarray(
                np.concatenate(
                    [shared_gate_up[:, sl],
                     shared_gate_up[:, 1024 + c * 128:1024 + (c + 1) * 128]],
                    axis=1).reshape(KT_H, 128, 256)).astype(np_dt),
            "wsd": np.ascontiguousarray(shared_down[sl, :]).astype(np_dt),
        })
    return in_maps, flat_col


# ---------------------------------------------------------------------------
# Entry point
# ---------------------------------------------------------------------------

def kernel(hidden_states, gate_w, e_bias, w_gate_up, w_down,
           shared_gate_up, shared_down):
    global LAST_RESULTS
    mode = _mode()
    x = np.ascontiguousarray(np.asarray(hidden_states, dtype=np.float32))
    gate_w = np.asarray(gate_w, dtype=np.float32)
    e_bias = np.asarray(e_bias, dtype=np.float32)
    w_gate_up = np.asarray(w_gate_up, dtype=np.float32)
    w_down = np.asarray(w_down, dtype=np.float32)
    shared_gate_up = np.asarray(shared_gate_up, dtype=np.float32)
    shared_down = np.asarray(shared_down, dtype=np.float32)

    topk_w, topk_ids = _host_routing(x, gate_w, e_bias)
    plan = _make_plan(topk_ids)

    nc = _get_program(plan["slot_rows"], mode)
    in_maps, flat_col = _make_in_maps(
        x, w_gate_up, w_down, shared_gate_up, shared_down,
        topk_ids, plan, mode)

    trace = bool(int(os.environ.get("KERNEL_TRACE", "0")))
    res = run_bass_kernel_spmd(
        nc, in_maps, list(range(N_CORES)), trace=trace,
        tmpdir=os.environ.get("KERNEL_TRACE_DIR") or None)
    LAST_RESULTS = res

    # host combine: routed gather-sum + shared partial sum
    Y = np.concatenate([res.results[c]["yr"].T for c in range(N_CORES)], axis=0)
    w_flat = (topk_w * SCALE).astype(np.float32).reshape(-1)
    out = (Y[flat_col.reshape(-1)] * w_flat[:, None]).reshape(T, TOPK, H).sum(1)

    shared = res.results[0]["ys"].copy()
    for c in range(1, N_CORES):
        shared += res.results[c]["ys"]
    out += shared.T
    return out.astype(np.float32)


# revision 7
# speedup vs baseline: 1.2956x; 1.2956x over previous
"""MegrezMoE MoE layer on 8 Trainium2 NeuronCores.

Strategy (expert-parallel, host-routed dispatch):
  - Host computes the (tiny) router: logits -> grouped top-k ids/weights,
    exactly mirroring the reference's noaux_tc selection.
  - 32 experts are assigned 4-per-core, balanced by routed-token count.
    Tokens are gathered per expert on the host (transposed: [H, rows],
    rows padded to a per-slot static capacity) so the device kernel is a
    fully static SPMD program: per expert slot, gate_up matmul ->
    silu*mul -> down matmul, streaming the expert weight bank from HBM
    exactly once per core.
  - The shared-expert MLP is tensor-parallel across the 8 cores (each
    core owns a 128-wide slice of the shared intermediate dim) and its
    partial outputs are summed on the host.
  - Matmuls run in bf16 with fp32 PSUM accumulation (KERNEL_DTYPE=f32r
    selects a float32r variant that keeps fp32 data in HBM).
  - Host combines: out[t] = sum_k w[t,k]*SCALE * y_col(t,k) + shared[t].

kernel() takes the full unsharded inputs, returns the full [1024, 2048]
fp32 output.
"""

import os

import ml_dtypes
import numpy as np

import concourse.bass as bass
import concourse.tile as tile
from concourse import bacc, mybir
from concourse.bass_utils import run_bass_kernel_spmd

# Model dims (hardcoded per problem spec)
H = 2048
E = 32
I = 1024
TOPK = 6
NGROUP = 8
TOPKG = 4
SCALE = 2.5
T = 1024

N_CORES = 8
EPC = 4          # experts per core
KT_H = H // 128  # 16 k-tiles over hidden dim
KT_I = I // 128  # 8 k-tiles over intermediate dim
WLOAD_K = 8      # k-tiles per weight DMA

F32 = mybir.dt.float32
F32R = mybir.dt.float32r
BF16 = mybir.dt.bfloat16

_PROGRAM_CACHE = {}
LAST_RESULTS = None  # BassKernelResults from the most recent run (for harness)


def _mode():
    return os.environ.get("KERNEL_DTYPE", "bf16")


# ---------------------------------------------------------------------------
# Host-side routing (mirrors reference._grouped_topk in fp32 numpy)
# ---------------------------------------------------------------------------

def _host_routing(x, gate_w, e_bias):
    logits = x @ gate_w                                   # [T, E] fp32
    scores = 1.0 / (1.0 + np.exp(-logits, dtype=np.float32))
    scores_choice = scores + e_bias[None, :]
    gsize = E // NGROUP
    grp = scores_choice.reshape(T, NGROUP, gsize)
    top2 = np.sort(grp, axis=-1)[:, :, -2:]
    group_scores = top2.sum(-1)                           # [T, G]
    gidx = np.argsort(-group_scores, axis=-1, kind="stable")[:, :TOPKG]
    gmask = np.zeros((T, NGROUP), bool)
    np.put_along_axis(gmask, gidx, True, axis=1)
    emask = np.repeat(gmask, gsize, axis=1)
    masked = np.where(emask, scores_choice, -np.inf)
    topk_ids = np.argsort(-masked, axis=-1, kind="stable")[:, :TOPK]
    topk_w = np.take_along_axis(scores, topk_ids, axis=1)
    topk_w = topk_w / topk_w.sum(-1, keepdims=True)
    return topk_w.astype(np.float32), topk_ids.astype(np.int64)


# ---------------------------------------------------------------------------
# Dispatch plan: expert -> (core, slot), per-slot static row capacities
# ---------------------------------------------------------------------------

def _make_plan(topk_ids):
    counts = np.bincount(topk_ids.ravel(), minlength=E)
    # slot capacity = max routed count in the slot's expert group, rounded
    # up to 8 (DMA alignment); capped per PSUM bank at 512
    padded = np.maximum(16, ((counts + 7) // 8) * 8)
    order = np.argsort(-padded, kind="stable")            # experts, big first
    expert_of = np.zeros((N_CORES, EPC), dtype=int)
    slot_rows = []
    for s in range(EPC):
        chunk = order[s * N_CORES:(s + 1) * N_CORES]
        for c in range(N_CORES):
            expert_of[c, s] = chunk[c]
        slot_rows.append(int(padded[chunk].max()))
    offs = np.concatenate([[0], np.cumsum(slot_rows)])
    return {
        "expert_of": expert_of,
        "slot_rows": tuple(slot_rows),
        "slot_offs": offs[:-1],
        "rtotal": int(offs[-1]),
        "counts": counts,
    }


# ---------------------------------------------------------------------------
# Bass program (SPMD; one program, per-core data)
# ---------------------------------------------------------------------------

def _build_program(slot_rows, mode):
    rtotal = sum(slot_rows)
    f32r = mode == "f32r"
    DTD = F32 if f32r else BF16      # dram dtype of matmul operands
    DTS = F32R if f32r else BF16     # sbuf dtype of matmul operands

    nc = bacc.Bacc("TRN2", target_bir_lowering=False, debug=False,
                   num_devices=N_CORES)

    # DRAM I/O (per core). Weight banks arrive host-reordered into k-slab
    # layout so every DMA is a large contiguous read:
    #   wgu: [EPC, 2(gate/up), KT_H, 128, I]
    #   wd:  [EPC, 2(out half), KT_I, 128, I]
    xg = nc.dram_tensor("xg", [H, rtotal], DTD, kind="ExternalInput")
    xf = nc.dram_tensor("xf", [H, T], DTD, kind="ExternalInput")
    wgu = nc.dram_tensor("wgu", [EPC, 2, KT_H, 128, I], DTD, kind="ExternalInput")
    wd = nc.dram_tensor("wd", [EPC, 2, KT_I, 128, I], DTD, kind="ExternalInput")
    wsg = nc.dram_tensor("wsg", [KT_H, 128, 256], DTD, kind="ExternalInput")
    wsd = nc.dram_tensor("wsd", [128, H], DTD, kind="ExternalInput")
    DTO = F32 if f32r else BF16
    yr = nc.dram_tensor("yr", [H, rtotal], DTO, kind="ExternalOutput")
    ys = nc.dram_tensor("ys", [H, T], DTO, kind="ExternalOutput")

    # casting DMA (fp32 dram -> f32r sbuf) must go via SWDGE
    ldma = nc.gpsimd.dma_start if f32r else nc.sync.dma_start

    def src3(ap2d, a):
        """DRAM [(a p), m] view as [p, a, m] for tiled SBUF loads."""
        return ap2d.rearrange("(a p) m -> a p m", p=128).transpose([1, 0, 2])

    def sb3(tile_ap, a):
        """SBUF [p, (a m)] view as [p, a, m]."""
        return tile_ap.rearrange("p (a m) -> p a m", a=a)

    CH = 256
    slot_offs = [0]
    for R in slot_rows[:-1]:
        slot_offs.append(slot_offs[-1] + R)

    with tile.TileContext(nc) as tc:
        with tc.tile_pool(name="psum", bufs=8, space="PSUM") as psum_pool, \
             tc.tile_pool(name="swg", bufs=1) as swg_pool, \
             tc.tile_pool(name="swd", bufs=1) as swd_pool, \
             tc.tile_pool(name="sxf", bufs=2) as sxf_pool, \
             tc.tile_pool(name="sact", bufs=2) as sact_pool, \
             tc.tile_pool(name="sout", bufs=2) as sout_pool, \
             tc.tile_pool(name="wsl", bufs=3) as w_pool, \
             tc.tile_pool(name="xs", bufs=2) as x_pool, \
             tc.tile_pool(name="gs", bufs=2) as g_pool, \
             tc.tile_pool(name="at", bufs=2) as a_pool, \
             tc.tile_pool(name="ost", bufs=2) as o_pool:

            wsg_sb = swg_pool.tile([128, KT_H * 256], DTS)
            wsd_sb = swd_pool.tile([128, H], DTS)

            def shared_weights():
                ldma(sb3(wsg_sb[:], KT_H), wsg.ap().transpose([1, 0, 2]))
                ldma(wsd_sb[:], wsd.ap())

            def shared_chunk(ch):
                xf_sb = sxf_pool.tile([128, KT_H * CH], DTS, name="xf_sb")
                ldma(sb3(xf_sb[:], KT_H),
                     src3(xf.ap()[:, ch * CH:(ch + 1) * CH], KT_H))
                ps_g = psum_pool.tile([128, CH], F32, tag="ps", name="ps_g")
                ps_u = psum_pool.tile([128, CH], F32, tag="ps", name="ps_u")
                for k in range(KT_H):
                    lg = wsg_sb[:, k * 256:k * 256 + 128]
                    lu = wsg_sb[:, k * 256 + 128:k * 256 + 256]
                    rx = xf_sb[:, k * CH:(k + 1) * CH]
                    nc.tensor.matmul(ps_g[:], lg, rx,
                                     start=(k == 0), stop=(k == KT_H - 1))
                    nc.tensor.matmul(ps_u[:], lu, rx,
                                     start=(k == 0), stop=(k == KT_H - 1))
                gss = sact_pool.tile([128, CH], F32, tag="sgs", name="gss")
                nc.scalar.activation(gss[:], ps_g[:],
                                     mybir.ActivationFunctionType.Sigmoid)
                nc.vector.tensor_mul(gss[:], gss[:], ps_g[:])
                a_s = sact_pool.tile([128, CH], DTS, tag="sas", name="a_s")
                nc.vector.tensor_mul(a_s[:], gss[:], ps_u[:])
                # down: 16 output m-tiles, single k (the 128-slice of I)
                for half in range(2):
                    stg = sout_pool.tile([128, 8 * CH], DTO, tag="sstg",
                                         name="stg")
                    for m in range(8):
                        pd = psum_pool.tile([128, CH], F32, tag="ps", name="pd")
                        lw = wsd_sb[:, (half * 8 + m) * 128:
                                    (half * 8 + m + 1) * 128]
                        nc.tensor.matmul(pd[:], lw, a_s[:],
                                         start=True, stop=True)
                        nc.scalar.copy(stg[:, m * CH:(m + 1) * CH], pd[:])
                    nc.sync.dma_start(
                        src3(ys.ap()[half * 1024:(half + 1) * 1024,
                                     ch * CH:(ch + 1) * CH], 8),
                        sb3(stg[:], 8))

            def expert_slot(s):
                R = slot_rows[s]
                off = slot_offs[s]
                xs = x_pool.tile([128, KT_H * R], DTS, tag="xs", name="xs")
                ldma(sb3(xs[:], KT_H), src3(xg.ap()[:, off:off + R], KT_H))

                gs = g_pool.tile([128, KT_I * R], F32, tag="gs", name="gs")
                at = a_pool.tile([128, KT_I * R], DTS, tag="at", name="at")

                for phase in range(2):  # 0 = gate, 1 = up
                    ps = [psum_pool.tile([128, R], F32, tag="ps", name="ps")
                          for _ in range(8)]
                    for kb in range(KT_H // WLOAD_K):
                        wt = w_pool.tile([128, WLOAD_K * I], DTS, tag="wsl",
                                         name="wt")
                        ldma(sb3(wt[:], WLOAD_K),
                             wgu.ap()[s, phase,
                                      kb * WLOAD_K:(kb + 1) * WLOAD_K]
                             .transpose([1, 0, 2]))
                        for kk in range(WLOAD_K):
                            k = kb * WLOAD_K + kk
                            rx = xs[:, k * R:(k + 1) * R]
                            for m in range(8):
                                lw = wt[:, kk * I + m * 128:
                                        kk * I + (m + 1) * 128]
                                nc.tensor.matmul(
                                    ps[m][:], lw, rx,
                                    start=(k == 0), stop=(k == KT_H - 1))
                    for m in range(8):
                        if phase == 0:
                            nc.scalar.activation(
                                gs[:, m * R:(m + 1) * R], ps[m][:],
                                mybir.ActivationFunctionType.Sigmoid)
                            nc.vector.tensor_mul(
                                gs[:, m * R:(m + 1) * R],
                                gs[:, m * R:(m + 1) * R], ps[m][:])
                        else:
                            nc.vector.tensor_mul(
                                at[:, m * R:(m + 1) * R],
                                gs[:, m * R:(m + 1) * R], ps[m][:])

                for half in range(2):
                    ps = [psum_pool.tile([128, R], F32, tag="ps", name="ps")
                          for _ in range(8)]
                    for kb in range(KT_I // WLOAD_K):
                        wt = w_pool.tile([128, WLOAD_K * I], DTS, tag="wsl",
                                         name="wt")
                        ldma(sb3(wt[:], WLOAD_K),
                             wd.ap()[s, half,
                                     kb * WLOAD_K:(kb + 1) * WLOAD_K]
                             .transpose([1, 0, 2]))
                        for kk in range(WLOAD_K):
                            k = kb * WLOAD_K + kk
                            ra = at[:, k * R:(k + 1) * R]
                            for m in range(8):
                                lw = wt[:, kk * I + m * 128:
                                        kk * I + (m + 1) * 128]
                                nc.tensor.matmul(
                                    ps[m][:], lw, ra,
                                    start=(k == 0), stop=(k == KT_I - 1))
                    stg = o_pool.tile([128, 8 * R], DTO, tag="ost", name="stg")
                    for m in range(8):
                        nc.scalar.copy(stg[:, m * R:(m + 1) * R], ps[m][:])
                    nc.sync.dma_start(
                        src3(yr.ap()[half * 1024:(half + 1) * 1024,
                                     off:off + R], 8),
                        sb3(stg[:], 8))

            # experts carry the bulk of the DMA stream; shared-expert
            # chunks are interleaved to fill PE gaps at phase boundaries
            shared_weights()
            expert_slot(0)
            shared_chunk(0)
            shared_chunk(1)
            expert_slot(1)
            shared_chunk(2)
            expert_slot(2)
            shared_chunk(3)
            expert_slot(3)

    nc.compile()
    return nc


def _get_program(slot_rows, mode):
    key = (tuple(slot_rows), mode)
    if key not in _PROGRAM_CACHE:
        _PROGRAM_CACHE[key] = _build_program(slot_rows, mode)
    return _PROGRAM_CACHE[key]


# ---------------------------------------------------------------------------
# Per-core input construction (host shard + reorder + cast)
# ---------------------------------------------------------------------------

def _make_in_maps(x, w_gate_up, w_down, shared_gate_up, shared_down,
                  topk_ids, plan, mode):
    rtotal = plan["rtotal"]
    offs = plan["slot_offs"]
    expert_of = plan["expert_of"]
    np_dt = np.float32 if mode == "f32r" else ml_dtypes.bfloat16

    tok_of = [np.where((topk_ids == e).any(axis=1))[0] for e in range(E)]
    flat_col = np.zeros((T, TOPK), dtype=np.int64)

    xT = np.ascontiguousarray(x.T).astype(np_dt)          # [H, T]
    wgu_r = w_gate_up.reshape(E, KT_H, 128, 2, I).transpose(0, 3, 1, 2, 4)
    wd_r = w_down.reshape(E, KT_I, 128, 2, I).transpose(0, 3, 1, 2, 4)

    in_maps = []
    for c in range(N_CORES):
        xg_c = np.zeros((H, rtotal), dtype=np_dt)
        for s in range(EPC):
            e = expert_of[c, s]
            toks = tok_of[e]
            xg_c[:, offs[s]:offs[s] + len(toks)] = xT[:, toks]
            col_base = c * rtotal + offs[s]
            for pos, t in enumerate(toks):
                for k in np.nonzero(topk_ids[t] == e)[0]:
                    flat_col[t, k] = col_base + pos
        sl = slice(c * 128, (c + 1) * 128)
        in_maps.append({
            "xg": xg_c,
            "xf": xT,
            "wgu": np.ascontiguousarray(wgu_r[expert_of[c]]).astype(np_dt),
            "wd": np.ascontiguousarray(wd_r[expert_of[c]]).astype(np_dt),
            "wsg": np.ascontiguousarray(
                np.concatenate(
                    [shared_gate_up[:, sl],
                     shared_gate_up[:, 1024 + c * 128:1024 + (c + 1) * 128]],
                    axis=1).reshape(KT_H, 128, 256)).astype(np_dt),
            "wsd": np.ascontiguousarray(shared_down[sl, :]).astype(np_dt),
        })
    return in_maps, flat_col


# ---------------------------------------------------------------------------
# Entry point
# ---------------------------------------------------------------------------

def kernel(hidden_states, gate_w, e_bias, w_gate_up, w_down,
           shared_gate_up, shared_down):
    global LAST_RESULTS
    mode = _mode()
    x = np.ascontiguousarray(np.asarray(hidden_states, dtype=np.float32))
    gate_w = np.asarray(gate_w, dtype=np.float32)
    e_bias = np.asarray(e_bias, dtype=np.float32)
    w_gate_up = np.asarray(w_gate_up, dtype=np.float32)
    w_down = np.asarray(w_down, dtype=np.float32)
    shared_gate_up = np.asarray(shared_gate_up, dtype=np.float32)
    shared_down = np.asarray(shared_down, dtype=np.float32)

    topk_w, topk_ids = _host_routing(x, gate_w, e_bias)
    plan = _make_plan(topk_ids)

    nc = _get_program(plan["slot_rows"], mode)
    in_maps, flat_col = _make_in_maps(
        x, w_gate_up, w_down, shared_gate_up, shared_down,
        topk_ids, plan, mode)

    trace = bool(int(os.environ.get("KERNEL_TRACE", "0")))
    res = run_bass_kernel_spmd(
        nc, in_maps, list(range(N_CORES)), trace=trace,
        tmpdir=os.environ.get("KERNEL_TRACE_DIR") or None)
    LAST_RESULTS = res

    # host combine: routed gather-sum + shared partial sum
    Y = np.concatenate(
        [np.asarray(res.results[c]["yr"], dtype=np.float32).T
         for c in range(N_CORES)], axis=0)
    w_flat = (topk_w * SCALE).astype(np.float32).reshape(-1)
    out = (Y[flat_col.reshape(-1)] * w_flat[:, None]).reshape(T, TOPK, H).sum(1)

    shared = np.asarray(res.results[0]["ys"], dtype=np.float32)
    for c in range(1, N_CORES):
        shared = shared + np.asarray(res.results[c]["ys"], dtype=np.float32)
    out += shared.T
    return out.astype(np.float32)
